# revision 2
# baseline (speedup 1.0000x reference)
"""Multi-head attention with QK-LayerNorm on 8 Trainium2 NeuronCores (v4).

Problem: B=2, S=F=2048, D=1024, H=16, HD=64 (fp32).
    q = LN_head(x_q @ Wq) * HD^-0.5 ; k = LN_head(x_k @ Wk) ; v = x_v @ Wv
    ctx = softmax(q k^T) v ; out = LN(ctx) @ Wproj

Sharding (8 cores, 2 groups of 4 by batch): core c owns batch c//4 and
row-slice c%4 (512 query rows, 512 kv rows), all-gathers kT/v within its
group, and computes attention + output projection for its rows.

v4 structure (HW-profiled):
  - host pre-transposes inputs to bf16; weights bf16;
  - q-projection FIRST, then k (+AllGather kT); the v-projection and its
    AllGather overlap the first attention pairs (pair-0 AV waits on v);
  - per pair: one QK+exp block, then a lagged AV block one pair behind,
    so the PE streams AV(j-1) while ACT streams exp(j);
  - softmax denominators are NOT applied in phase 2: raw context rows and
    denominator rows are staged to SBUF (bf16) and normalization happens
    once per D-chunk in phase 3 (reciprocal + rank-1 PE broadcast folded
    into the LN scale);
  - QK-LN stats via tensor_reduce + ACT Square; rstd = exp(-0.5*ln(var))
    keeps the scalar engine on a single activation table set.
"""

import numpy as np

import concourse.bass as bass
import concourse.mybir as mybir
import concourse.tile as tile
from concourse import bacc, bass_utils
from concourse.masks import make_identity

F32 = mybir.dt.float32
F32R = mybir.dt.float32r
BF16 = mybir.dt.bfloat16
AF = mybir.ActivationFunctionType
ALU = mybir.AluOpType

B, S, F, D, H, HD = 2, 2048, 2048, 1024, 16, 64
EPS = 1e-5
NCORES = 8
GP = 4
SL = S // GP                # 512 local query rows
FL = F // GP                # 512 local kv rows
KC = D // 128               # 8 D-chunks
MC = SL // 128              # 4 local row chunks
NPAIR = H // 2
GROUPS = [[0, 1, 2, 3], [4, 5, 6, 7]]


def _dma_big(nc, out, in_):
    return nc.sync.dma_start(out=out, in_=in_)


def _dma_small(nc, out, in_):
    return nc.gpsimd.dma_start(out=out, in_=in_)


def build(n_repeat=1, fastln=True, rep_scope="all", att_mode="full"):
    nc = bacc.Bacc(None, target_bir_lowering=False)

    xqT = nc.declare_dram_parameter("xqT", [D, SL], BF16, isOutput=False)
    xkT = nc.declare_dram_parameter("xkT", [D, FL], BF16, isOutput=False)
    xvT = nc.declare_dram_parameter("xvT", [D, FL], BF16, isOutput=False)
    wq = nc.declare_dram_parameter("wq", [D, D], BF16, isOutput=False)
    wk = nc.declare_dram_parameter("wk", [D, D], BF16, isOutput=False)
    wv = nc.declare_dram_parameter("wv", [D, D], BF16, isOutput=False)
    wp = nc.declare_dram_parameter("wp", [D, D], BF16, isOutput=False)
    qg2 = nc.declare_dram_parameter("qg2", [128], F32, isOutput=False)
    qb2 = nc.declare_dram_parameter("qb2", [128], F32, isOutput=False)
    kg2 = nc.declare_dram_parameter("kg2", [128], F32, isOutput=False)
    kb2 = nc.declare_dram_parameter("kb2", [128], F32, isOutput=False)
    og = nc.declare_dram_parameter("og", [D], F32, isOutput=False)
    ob = nc.declare_dram_parameter("ob", [D], F32, isOutput=False)
    out = nc.declare_dram_parameter("out", [SL, D], F32, isOutput=True)

    with tile.TileContext(nc) as tc:
        with (
            tc.tile_pool(name="const", bufs=1) as const,
            tc.tile_pool(name="persist", bufs=1) as persist,
            tc.tile_pool(name="dram", bufs=1, space="DRAM") as dram,
        ):
            ident = const.tile([128, 128], F32)
            make_identity(nc, ident[:, :])
            ones_f = const.tile([128, 1], F32)
            nc.vector.memset(ones_f, 1.0)
            ones_col = const.tile([128, 1], F32R)
            nc.vector.tensor_copy(ones_col, ones_f)
            eps_t = const.tile([128, 1], F32)
            nc.vector.memset(eps_t, EPS)
            ones_row_f = const.tile([1, 128], F32)
            nc.vector.memset(ones_row_f, 1.0)
            ones_row = const.tile([1, 128], F32R)
            nc.vector.tensor_copy(ones_row, ones_row_f)
            # per-kc selectors sel16[kc]: [16,128] with row 2kc ones on
            # partitions 0-63 and row 2kc+1 ones on 64-127, so
            # sel16[kc].T @ den16 broadcasts each head's denominator onto
            # its 64 d-partitions.  (memset/DVE can't address odd base
            # partitions; rows are built at partition 0 and DMA'd in.)
            selr0 = const.tile([1, 128], F32R)
            nc.vector.memset(selr0.bitcast(F32), 0.0)
            nc.vector.memset(selr0.bitcast(F32)[0:1, 0:64], 1.0)
            selr1 = const.tile([1, 128], F32R)
            nc.vector.memset(selr1.bitcast(F32), 0.0)
            nc.vector.memset(selr1.bitcast(F32)[0:1, 64:128], 1.0)
            sel_all = const.tile([16, KC, 128], F32R, tag="sel16")
            nc.vector.memset(sel_all.bitcast(F32), 0.0)
            for r in range(16):
                _dma_small(nc, sel_all[r:r + 1, r // 2, :],
                           selr0 if r % 2 == 0 else selr1)
            sel16 = [sel_all[:, kc, :] for kc in range(KC)]

            def col128(param):
                t = const.tile([128, 1], F32, tag=f"c_{param.name}")
                _dma_small(nc, t, param[:, None])
                return t

            qg2_t = col128(qg2)
            qb2_t = col128(qb2)
            kg2_t = col128(kg2)
            kb2_t = col128(kb2)
            og_pp = const.tile([128, KC], F32)
            _dma_small(nc, og_pp, og.rearrange("(kc p) -> p kc", p=128))
            ob_pp = const.tile([128, KC], F32)
            _dma_small(nc, ob_pp, ob.rearrange("(kc p) -> p kc", p=128))

            for _rep in range(n_repeat if rep_scope == "all" else 1):
                qT = persist.tile([128, KC, SL], BF16, tag="qT", name="qT")
                ctxT = persist.tile([128, KC, SL], F32R, tag="ctxT",
                                    name="ctxT")
                den_all = persist.tile([16, SL], F32, tag="den", name="den")

                kT_bounce = dram.tile([D, FL], BF16, tag="kTb", name="kTb")
                v_bounce = dram.tile([FL, D], BF16, tag="vb", name="vb")
                kT_all = dram.tile([GP, D, FL], BF16, tag="kTa", name="kTa")
                v_all = dram.tile([GP, FL, D], BF16, tag="va", name="va")

                # ================= phase 1: projections =================
                _wpool_cm = tc.tile_pool(name="wpool", bufs=3)
                wpool = _wpool_cm.__enter__()
                _xpool_cm = tc.tile_pool(name="xpool", bufs=3)
                xpool = _xpool_cm.__enter__()

                def load_w(wparam):
                    w = wpool.tile([128, KC, D], BF16, tag="w")
                    _dma_big(nc, w,
                             wparam.rearrange("(kc p) n -> p kc n", p=128))
                    return w

                def load_xT(xparam):
                    t = xpool.tile([128, KC, FL], BF16, tag="xT")
                    _dma_big(nc, t,
                             xparam.rearrange("(kc p) s -> p kc s", p=128))
                    return t

                def proj_chain(xT, w, which, ps_bufs=3):
                    with (
                        tc.tile_pool(name=f"p1_{which}", bufs=2) as p1,
                        tc.tile_pool(name=f"p1s_{which}", bufs=2) as p1s,
                        tc.tile_pool(name=f"p1ps_{which}", bufs=ps_bufs,
                                     space="PSUM") as p1ps,
                        tc.tile_pool(name=f"p1tp_{which}", bufs=2,
                                     space="PSUM") as p1tp,
                    ):
                        def mm_stage(m):
                            ps = p1ps.tile([128, D], F32, tag="nat",
                                           name="nat")
                            for n in range(2):
                                for kc in range(KC):
                                    nc.tensor.matmul(
                                        ps[:, n * 512:(n + 1) * 512],
                                        xT[:, kc, m * 128:(m + 1) * 128],
                                        w[:, kc, n * 512:(n + 1) * 512],
                                        start=(kc == 0), stop=(kc == KC - 1),
                                    )
                            return ps

                        def ln_stage(m, ps):
                            psv = ps[:, :].rearrange("p (h d) -> p h d", d=HD)
                            sq = p1.tile([128, D], F32, tag="sq", name="sq")
                            nc.scalar.activation(sq, ps[:, :], AF.Square)
                            s1 = p1s.tile([128, H], F32, tag="s1")
                            nc.vector.tensor_reduce(
                                s1, psv, mybir.AxisListType.X, ALU.add)
                            s2 = p1s.tile([128, H], F32, tag="s2")
                            nc.vector.tensor_reduce(
                                s2, sq.rearrange("p (h d) -> p h d", d=HD),
                                mybir.AxisListType.X, ALU.add)
                            mean = p1s.tile([128, H], F32, tag="mean")
                            nc.vector.tensor_scalar_mul(mean, s1, 1.0 / HD)
                            var = p1s.tile([128, H], F32, tag="var")
                            nc.vector.tensor_scalar_mul(var, s2, 1.0 / HD)
                            m2 = p1s.tile([128, H], F32, tag="m2")
                            nc.vector.tensor_mul(m2, mean, mean)
                            nc.vector.tensor_sub(var, var, m2)
                            rstd = p1s.tile([128, H], F32, tag="rstd")
                            nc.scalar.activation(rstd, var, AF.Ln,
                                                 bias=eps_t[:, :])
                            nc.scalar.activation(rstd, rstd, AF.Exp,
                                                 scale=-0.5)
                            ln = p1.tile([128, H, HD], F32, tag="ln",
                                         name="ln")
                            if which == "q" and fastln:
                                nc.vector.tensor_mul(
                                    ln, psv,
                                    rstd[:, :, None].broadcast_to(
                                        [128, H, HD]))
                            else:
                                nc.vector.tensor_sub(
                                    ln, psv,
                                    mean[:, :, None].broadcast_to(
                                        [128, H, HD]))
                                nc.vector.tensor_mul(
                                    ln, ln,
                                    rstd[:, :, None].broadcast_to(
                                        [128, H, HD]))
                            return ln.rearrange("p h d -> p (h d)")

                        def post_stage(m, ps):
                            if which == "v":
                                v_sb = p1.tile([128, D], BF16, tag="vout",
                                               name="v_sb")
                                nc.vector.tensor_copy(v_sb, ps[:, :])
                                _dma_big(nc,
                                         v_bounce[m * 128:(m + 1) * 128, :],
                                         v_sb)
                                return
                            g_t, b_t = ((qg2_t, qb2_t) if which == "q"
                                        else (kg2_t, kb2_t))
                            lnf = ln_stage(m, ps)
                            for half in range(2):
                                psT = p1tp.tile([128, 512], F32, tag="tp",
                                                name="psT")
                                for qq in range(4):
                                    kc = half * 4 + qq
                                    nc.tensor.transpose(
                                        psT[:, qq * 128:(qq + 1) * 128],
                                        lnf[:, kc * 128:(kc + 1) * 128],
                                        ident[:, :])
                                psTv = psT[:, :].rearrange(
                                    "p (q j) -> p q j", j=128)
                                if which == "q":
                                    nc.vector.tensor_scalar(
                                        out=qT[:, half * 4:(half + 1) * 4,
                                               m * 128:(m + 1) * 128],
                                        in0=psTv,
                                        scalar1=g_t[:, :], scalar2=b_t[:, :],
                                        op0=ALU.mult, op1=ALU.add)
                                else:
                                    kTl = p1.tile([128, 4, 128], BF16,
                                                  tag="kTl", name="kTl")
                                    nc.vector.tensor_scalar(
                                        out=kTl, in0=psTv,
                                        scalar1=g_t[:, :], scalar2=b_t[:, :],
                                        op0=ALU.mult, op1=ALU.add)
                                    _dma_big(
                                        nc,
                                        kT_bounce.rearrange(
                                            "(kc p) s -> p kc s", p=128)
                                        [:, half * 4:(half + 1) * 4,
                                         m * 128:(m + 1) * 128],
                                        kTl)

                        prev = None
                        for m in range(MC):
                            ps = mm_stage(m)
                            if prev is not None:
                                post_stage(m - 1, prev)
                            prev = ps
                        post_stage(MC - 1, prev)

                # q first, then k (+AG), then v (overlapping attention)
                xq_t = load_xT(xqT)
                w_q = load_w(wq)
                xk_t = load_xT(xkT)
                w_k = load_w(wk)
                xv_t = load_xT(xvT)
                proj_chain(xq_t, w_q, "q", ps_bufs=3)
                w_v = load_w(wv)
                proj_chain(xk_t, w_k, "k", ps_bufs=3)
                nc.gpsimd.collective_compute(
                    "AllGather", ALU.bypass, replica_groups=GROUPS,
                    ins=[kT_bounce.opt()], outs=[kT_all.opt()],
                )
                proj_chain(xv_t, w_v, "v", ps_bufs=1)
                nc.gpsimd.collective_compute(
                    "AllGather", ALU.bypass, replica_groups=GROUPS,
                    ins=[v_bounce.opt()], outs=[v_all.opt()],
                )
                _xpool_cm.__exit__(None, None, None)
                w_p = load_w(wp)

                # ================= phase 2: attention =================
                with (
                    tc.tile_pool(name="vext", bufs=1) as vextp,
                    tc.tile_pool(name="pt", bufs=20) as ptp,
                    tc.tile_pool(name="att", bufs=3) as att,
                    tc.tile_pool(name="kpair", bufs=2) as kpairp,
                    tc.tile_pool(name="att_ps", bufs=2, space="PSUM") as att_ps,
                    tc.tile_pool(name="ctx_ps", bufs=1, space="PSUM") as ctx_psp,
                ):
                    v_ext = []
                    pair_pts = {}
                    pair_ctx = {}

                    def load_v_ext():
                        for jj in range(F // 128):
                            g, lj = jj // 4, jj % 4
                            vt = vextp.tile([128, H, HD + 1], BF16,
                                            tag=f"vext{jj}")
                            _dma_big(
                                nc, vt[:, :, 1:HD + 1],
                                v_all[g, lj * 128:(lj + 1) * 128, :]
                                .rearrange("p (h d) -> p h d", d=HD))
                            nc.vector.tensor_copy(
                                vt[:, :, 0:1],
                                ones_f[:, None, :].broadcast_to([128, H, 1]))
                            v_ext.append(vt)

                    def load_kT(j):
                        kT_pair = kpairp.tile([128, F], BF16, tag="kp")
                        for g in range(GP):
                            _dma_big(
                                nc, kT_pair[:, g * FL:(g + 1) * FL],
                                kT_all[g, j * 128:(j + 1) * 128, :])
                        return kT_pair

                    def qk_sc(j, kT_pair, sc):
                        """QK matmuls + exp for super-chunk sc of pair j."""
                        sp = [None, None]
                        for hh in range(2):
                            sp[hh] = att_ps.tile([128, 1024], F32,
                                                 tag="sp", name="sp")
                        for cc in range(2):
                            fc = sc * 2 + cc
                            for hh in range(2):
                                nc.tensor.matmul(
                                    sp[hh][:, cc * 512:(cc + 1) * 512],
                                    kT_pair[hh * 64:(hh + 1) * 64,
                                            fc * 128:(fc + 1) * 128],
                                    qT[hh * 64:(hh + 1) * 64, j, :],
                                    start=True, stop=True,
                                    tile_position=(hh * 64, 0),
                                )
                        for hh in range(2):
                            pt = ptp.tile([128, 1024], BF16,
                                          tag="pt", name="pt")
                            nc.scalar.activation(pt, sp[hh][:, :], AF.Exp)
                            pair_pts[j].append(pt)

                    def av_sc(j, sc):
                        """AV accumulation for super-chunk sc of pair j."""
                        if sc == 0:
                            pair_ctx[j] = [
                                ctx_psp.tile([HD + 1, SL], F32, tag="ctxA",
                                             name="ctxA"),
                                ctx_psp.tile([HD + 1, SL], F32, tag="ctxB",
                                             name="ctxB"),
                            ]
                        ctx_ps = pair_ctx[j]
                        pts = pair_pts[j]
                        for cc in range(2):
                            fc = sc * 2 + cc
                            for hh in range(2):
                                nc.tensor.matmul(
                                    ctx_ps[hh][:, :],
                                    v_ext[fc][:, 2 * j + hh, :],
                                    pts[2 * sc + hh]
                                    [:, cc * 512:(cc + 1) * 512],
                                    start=(att_mode == "avnoacc"
                                           or (sc == 0 and cc == 0)),
                                    stop=(att_mode == "avnoacc"
                                          or (sc == 7 and cc == 1)),
                                )

                    def finish_pair(j):
                        ctx_ps = pair_ctx.pop(j)
                        pair_pts.pop(j)
                        for hh in range(2):
                            uT = att.tile([HD + 1, SL], F32, tag="uT")
                            nc.vector.tensor_copy(uT, ctx_ps[hh][:, :])
                            _dma_big(nc, ctxT[hh * 64:(hh + 1) * 64, j, :],
                                     uT[1:HD + 1, :].bitcast(F32R))
                            _dma_small(
                                nc, den_all[2 * j + hh:2 * j + hh + 1, :],
                                uT[0:1, :])

                    # lag-1 software pipeline, interleaved per super-chunk:
                    # the PE queue alternates QK(j,sc) with ready AV(j-1,sc)
                    # so a QK waiting on an sp slot never starves the PE.
                    # Pair 0's kT DMA precedes the v_ext loads so it isn't
                    # queued behind the AllGather-v dependency.
                    kp = load_kT(0)
                    pair_pts[0] = []
                    load_v_ext()
                    for sc in range(8):
                        qk_sc(0, kp, sc)
                    for j in range(1, NPAIR):
                        kp = load_kT(j)
                        pair_pts[j] = []
                        for sc in range(8):
                            qk_sc(j, kp, sc)
                            av_sc(j - 1, sc)
                        finish_pair(j - 1)
                    for sc in range(8):
                        av_sc(NPAIR - 1, sc)
                    finish_pair(NPAIR - 1)

                # ================= phase 3: out-LN + projection =================
                with (
                    tc.tile_pool(name="p3", bufs=2) as p3,
                    tc.tile_pool(name="p3w", bufs=1) as p3w,
                    tc.tile_pool(name="p3s", bufs=1) as p3s,
                    tc.tile_pool(name="bc_ps", bufs=2, space="PSUM") as bc_psp,
                    tc.tile_pool(name="st_ps", bufs=1, space="PSUM") as st_ps,
                    tc.tile_pool(name="o_ps", bufs=2, space="PSUM") as o_ps,
                ):
                    wproj = w_p
                    denr = persist.tile([16, SL], F32R, tag="denr")
                    with nc.allow_low_precision(
                            reason="fp32r is 32-bit storage"):
                        nc.vector.reciprocal(denr, den_all[:, :])

                    # normalized context (f32r) per D-chunk
                    ctxN = p3w.tile([128, KC, SL], F32R, tag="ctxN")
                    sum_ps = st_ps.tile([1, SL], F32, tag="sum")
                    sq_ps = st_ps.tile([1, SL], F32, tag="sqs")
                    for kc in range(KC):
                        bc = bc_psp.tile([128, SL], F32, tag="bc")
                        nc.tensor.matmul(bc[:, :], sel16[kc], denr[:, :],
                                         start=True, stop=True)
                        nc.vector.tensor_mul(
                            ctxN[:, kc, :], ctxT[:, kc, :].bitcast(F32),
                            bc[:, :])
                        nc.tensor.matmul(sum_ps[:, :], ones_col,
                                         ctxN[:, kc, :],
                                         start=(kc == 0), stop=(kc == KC - 1))
                        sq = p3.tile([128, SL], F32R, tag="sq")
                        nc.vector.tensor_mul(
                            sq, ctxN[:, kc, :].bitcast(F32),
                            ctxN[:, kc, :].bitcast(F32))
                        nc.tensor.matmul(sq_ps[:, :], ones_col, sq,
                                         start=(kc == 0), stop=(kc == KC - 1))

                    mean = p3s.tile([1, SL], F32, tag="mean")
                    nc.vector.tensor_scalar_mul(mean, sum_ps[:, :], 1.0 / D)
                    var = p3s.tile([1, SL], F32, tag="var")
                    nc.vector.tensor_scalar_mul(var, sq_ps[:, :], 1.0 / D)
                    m2 = p3s.tile([1, SL], F32, tag="m2")
                    nc.vector.tensor_mul(m2, mean, mean)
                    nc.vector.tensor_sub(var, var, m2)
                    rstd = p3s.tile([1, SL], F32, tag="rstd")
                    nc.scalar.activation(rstd, var, AF.Ln, bias=eps_t[0:1, :])
                    nc.scalar.activation(rstd, rstd, AF.Exp, scale=-0.5)
                    negm = p3s.tile([1, SL], F32, tag="negm")
                    nc.vector.tensor_mul(negm, mean, rstd)
                    nc.vector.tensor_scalar_mul(negm, negm, -1.0)
                    rstd_r = p3s.tile([1, SL], F32R, tag="rstd_r")
                    nc.vector.tensor_copy(rstd_r, rstd)
                    negm_r = p3s.tile([1, SL], F32R, tag="negm_r")
                    nc.vector.tensor_copy(negm_r, negm)
                    bc2 = bc_psp.tile([128, SL], F32, tag="bc")
                    nc.tensor.matmul(bc2[:, :], ones_row, rstd_r,
                                     start=True, stop=True)
                    rstd_b = p3s.tile([128, SL], F32, tag="rstd_b")
                    nc.vector.tensor_copy(rstd_b, bc2[:, :])
                    bc3 = bc_psp.tile([128, SL], F32, tag="bc")
                    nc.tensor.matmul(bc3[:, :], ones_row, negm_r,
                                     start=True, stop=True)
                    negm_b = p3s.tile([128, SL], F32, tag="negm_b")
                    nc.vector.tensor_copy(negm_b, bc3[:, :])

                    ctxn = p3w.tile([128, KC, SL], BF16, tag="ctxn")
                    for kc in range(KC):
                        t = p3.tile([128, SL], F32, tag="lnt")
                        nc.vector.tensor_mul(
                            t, ctxN[:, kc, :].bitcast(F32), rstd_b[:, :])
                        nc.vector.tensor_add(t, t, negm_b[:, :])
                        nc.vector.tensor_scalar(
                            out=ctxn[:, kc, :], in0=t,
                            scalar1=og_pp[:, kc:kc + 1],
                            scalar2=ob_pp[:, kc:kc + 1],
                            op0=ALU.mult, op1=ALU.add)

                    for m in range(MC):
                        ps = o_ps.tile([128, D], F32, tag="o")
                        for n in range(2):
                            for kc in range(KC):
                                nc.tensor.matmul(
                                    ps[:, n * 512:(n + 1) * 512],
                                    ctxn[:, kc, m * 128:(m + 1) * 128],
                                    wproj[:, kc, n * 512:(n + 1) * 512],
                                    start=(kc == 0), stop=(kc == KC - 1),
                                )
                        o_sb = p3.tile([128, D], F32, tag="osb")
                        nc.vector.tensor_copy(o_sb, ps[:, :])
                        _dma_big(nc, out[m * 128:(m + 1) * 128, :], o_sb)

                _wpool_cm.__exit__(None, None, None)
    nc.finalize()
    return nc


_NC_CACHE = {}


def _get_nc(fastln):
    key = bool(fastln)
    if key not in _NC_CACHE:
        _NC_CACHE[key] = build(1, fastln=key)
    return _NC_CACHE[key]


def _fastln_ok(q_gamma, q_beta, k_gamma, k_beta):
    qg = np.asarray(q_gamma, np.float64)
    qb = np.asarray(q_beta, np.float64)
    kg = np.asarray(k_gamma, np.float64)
    kb = np.asarray(k_beta, np.float64)
    gg = qg * kg
    return (np.allclose(qb, 0.0, atol=1e-12)
            and np.ptp(gg) < 1e-12 * max(1.0, np.abs(gg).max())
            and abs(np.sum(qg * kb)) < 1e-9)


def _prep_common(Wq, Wk, Wv, Wproj, q_gamma, q_beta, k_gamma, k_beta,
                 out_gamma, out_beta):
    import ml_dtypes
    bf16 = ml_dtypes.bfloat16
    scale = np.float64(HD) ** -0.5

    def w_bf(w):
        return np.ascontiguousarray(np.asarray(w, np.float32).astype(bf16))

    def tile2(v, s=1.0):
        v = np.asarray(v, np.float64) * s
        return np.ascontiguousarray(np.tile(v, 2).astype(np.float32))

    return {
        "wq": w_bf(Wq), "wk": w_bf(Wk), "wv": w_bf(Wv), "wp": w_bf(Wproj),
        "qg2": tile2(q_gamma, scale), "qb2": tile2(q_beta, scale),
        "kg2": tile2(k_gamma), "kb2": tile2(k_beta),
        "og": np.ascontiguousarray(np.asarray(out_gamma, np.float32)),
        "ob": np.ascontiguousarray(np.asarray(out_beta, np.float32)),
    }


def make_in_maps(inputs):
    import ml_dtypes
    bf16 = ml_dtypes.bfloat16
    common = _prep_common(
        inputs["Wq"], inputs["Wk"], inputs["Wv"], inputs["Wproj"],
        inputs["q_gamma"], inputs["q_beta"], inputs["k_gamma"],
        inputs["k_beta"], inputs["out_gamma"], inputs["out_beta"])
    x_q = np.asarray(inputs["x_q"], np.float32)
    x_k = np.asarray(inputs["x_k"], np.float32)
    x_v = np.asarray(inputs["x_v"], np.float32)
    in_maps = []
    for c in range(NCORES):
        b, r = c // GP, c % GP
        in_maps.append({
            "xqT": np.ascontiguousarray(
                x_q[b, r * SL:(r + 1) * SL, :].T.astype(bf16)),
            "xkT": np.ascontiguousarray(
                x_k[b, r * FL:(r + 1) * FL, :].T.astype(bf16)),
            "xvT": np.ascontiguousarray(
                x_v[b, r * FL:(r + 1) * FL, :].T.astype(bf16)),
            **common,
        })
    return in_maps


def kernel(x_q, x_k, x_v, Wq, Wk, Wv, Wproj,
           q_gamma, q_beta, k_gamma, k_beta, out_gamma, out_beta,
           _trace=False):
    inputs = {
        "x_q": x_q, "x_k": x_k, "x_v": x_v, "Wq": Wq, "Wk": Wk, "Wv": Wv,
        "Wproj": Wproj, "q_gamma": q_gamma, "q_beta": q_beta,
        "k_gamma": k_gamma, "k_beta": k_beta, "out_gamma": out_gamma,
        "out_beta": out_beta,
    }
    in_maps = make_in_maps(inputs)
    fast = _fastln_ok(q_gamma, q_beta, k_gamma, k_beta)
    nc = _get_nc(fast)
    res = bass_utils.run_bass_kernel_spmd(
        nc, in_maps, list(range(NCORES)), trace=_trace)
    full = np.empty((B, S, D), dtype=np.float32)
    for c in range(NCORES):
        b, r = c // GP, c % GP
        full[b, r * SL:(r + 1) * SL, :] = res.results[c]["out"]
    if _trace:
        return full, res
    return full
